# revision 21
# baseline (speedup 1.0000x reference)
"""Trainium2 Bass kernel for DirectVolumeRenderer (axis-aligned camera).

Factorization (per depth p, camera R=I so sample coords are separable):
    trilinear(vol) = z-lerp of 2 slices -> two matmuls with the SAME tent
    matrix  A_p[v,q] = relu(1 - |v - (a_p + s_p*q)|):
        T1   = Zp^T @ A_p          (contract y)
        feat = A_p^T @ T1          (contract x) -> image in [px,py] layout
    sigma_p = 0.1*az_p * av_p[px] (x) av_p[py]  (rank-1, host vectors)

Key simplification: transmittance Gamma_k is DATA-INDEPENDENT (density is
a constant 0.1 and the ray/volume geometry is fixed).  On sigma_k's
support (the nested valid square S_k) every earlier sigma_j was fully
inside its own square, so Gamma_k == gamma_k = prod_{j<k}(1 - 0.1*az_j),
a host-computable SCALAR (validated to ~3e-6 against the exact 2D
recurrence).  The device therefore computes only
    rgb = sum_k (gamma_k * sigma_k) .* feat_k
with gamma_k folded into the host-side sigma u-vectors -- no serial
compositing chain on the device at all.

Sharding: 240 active depths split into 8 contiguous runs of 30 per core.
Cross-core combine is one fp16 AllReduce(sum) + normalization.

Engines per depth: PE does sigma outer-product (f32r), mm1/mm2 (bf16) and
the rgb PSUM accumulation (bf16 identity matmul); ACT builds the tent and
the PSUM->SBUF T1 copy (scaled by -wz_large); DVE does the one-op z-lerp
(fp8 slices -> bf16) and the weight multiply.  Slab DMA is prefetched
in-loop (ring buffer) so compute chases the stream.
"""
import os
import sys
import numpy as np

for _p in ("/opt/trn_rl_repo", "/root/.axon_site/_ro/trn_rl_repo"):
    if os.path.isdir(_p) and _p not in sys.path:
        sys.path.insert(0, _p)

IMG = 256
NPTS = 320
MIN_D, MAX_D = 2.0, 6.0
FOCAL = 2.0
DENSITY = 0.1
EPS = 1e-8
N_CORES = 8


# ----------------------------------------------------------------------------
# host-side geometry
# ----------------------------------------------------------------------------

def _geometry(T):
    """Per-depth separable sampling params (f64). Requires R=I and Tx==Ty."""
    Tx, Ty, Tz = float(T[0]), float(T[1]), float(T[2])
    vox = 3.0 / 256.0
    half = vox * 255.0 * 0.5
    depths = np.linspace(MIN_D, MAX_D, NPTS)
    c = depths * 127.5 / (2.0 * half)
    s = c * (2.0 / 255.0)
    a = 127.5 - c - Tx * 127.5 / half
    iz = 127.5 * ((depths - Tz) / half + 1.0)
    z0 = np.floor(iz).astype(np.int64)
    fz = iz - z0
    z1 = z0 + 1
    wz0 = np.where((z0 >= 0) & (z0 < 256), 1.0 - fz, 0.0)
    wz1 = np.where((z1 >= 0) & (z1 < 256), fz, 0.0)
    az = wz0 + wz1
    q = np.arange(IMG)
    ic = a[:, None] + s[:, None] * q[None, :]
    c0 = np.floor(ic)
    fc = ic - c0
    av = (np.where((c0 >= 0) & (c0 < 256), 1.0 - fc, 0.0)
          + np.where((c0 + 1 >= 0) & (c0 + 1 < 256), fc, 0.0))
    return dict(s=s, a=a, z0=z0, z1=z1, wz0=wz0, wz1=wz1, az=az, av=av,
                active=az > 0)


def _host_inputs(vol, T):
    """Build the 8 per-core input maps. vol: (256,256,256) f32 (z,y,x)."""
    import ml_dtypes
    g = _geometry(T)
    act = np.nonzero(g["active"])[0]

    # gamma_k = prod_{j<k} (1 - 0.1*az_j): global transmittance scalars
    cfac = 1.0 - DENSITY * g["az"]
    gam = np.ones(NPTS)
    gam[1:] = np.cumprod(cfac)[:-1]
    # truncate depths whose remaining transmittance is negligible
    # (gamma < 4e-5 -> contribution ~1e-4 of the image; validated 5.7e-5
    # rel err at 96 of 240 depths)
    act = np.array([p for p in act if gam[p] > 4e-5])
    nd = int(np.ceil(len(act) / N_CORES))
    # fold gamma into the (negative) sigma u-vector
    uneg_all = (-DENSITY * (gam * g["az"])[:, None] * g["av"])
    v_all = g["av"]

    vol16 = vol.astype(ml_dtypes.bfloat16)
    in_maps = []
    for cidx in range(N_CORES):
        ks = [int(act[i]) for i in range(cidx * nd, min((cidx + 1) * nd, len(act)))]

        slices = np.zeros((128, nd, 1024), ml_dtypes.bfloat16)
        wlp = np.zeros((128, nd), np.float32)
        vbs = np.zeros((128, nd, 512), ml_dtypes.bfloat16)
        tents = np.zeros((128, nd, 512), ml_dtypes.bfloat16)
        qrow = np.arange(IMG, dtype=np.float64)
        vgrid = np.arange(256, dtype=np.float64)

        for j, p in enumerate(ks):
            w0, w1 = g["wz0"][p], g["wz1"][p]
            zz0 = min(max(int(g["z0"][p]), 0), 255)
            zz1 = min(max(int(g["z1"][p]), 0), 255)
            if w0 <= w1:
                z_small, z_large, w_small, w_large = zz0, zz1, w0, w1
            else:
                z_small, z_large, w_small, w_large = zz1, zz0, w1, w0
            # slot0 = (w_small/w_large)-prescaled small slice, slot1 = large
            r = np.float32(w_small / w_large)
            for si, zz, sc in ((0, z_small, r), (1, z_large, np.float32(1.0))):
                sl = (vol16[zz].astype(np.float32) * sc).astype(vol16.dtype)
                slices[:, j, si * 512:(si + 1) * 512] = \
                    sl.reshape(2, 128, 256).transpose(1, 0, 2).reshape(128, 512)
            wlp[:, j] = np.float32(-w_large)
            # tent matrix A[v, q] = relu(1 - |v - ic(q)|), v = 128b + part
            ic = g["a"][p] + g["s"][p] * qrow
            A = np.clip(1.0 - np.abs(vgrid[:, None] - ic[None, :]), 0.0, None)
            for b in (0, 1):
                tents[:, j, 256 * b:256 * (b + 1)] = A[128 * b:128 * (b + 1)]
                # sigma field (gamma folded, negative): vb[p,256b+py]
                vbs[:, j, 256 * b:256 * (b + 1)] = np.outer(
                    uneg_all[p][128 * b:128 * (b + 1)], v_all[p])

        in_maps.append({
            "slices": slices.reshape(128, nd * 1024),
            "wlp": wlp,
            "vbs": vbs.reshape(128, nd * 512),
            "tents": tents.reshape(128, nd * 512),
            "identh": np.eye(128, dtype=ml_dtypes.bfloat16),
            "identf": np.eye(128, dtype=np.float32),
            "ones1": np.ones((1, 128), np.float32),
        })
    return in_maps, nd


# ----------------------------------------------------------------------------
# device program
# ----------------------------------------------------------------------------

_NC_CACHE = {}


def _build_nc(nd, sim=False):
    """sim=True replaces the AllReduce with a local DMA copy so the
    single-core TimelineSim cost model can run the program."""
    import concourse.bass as bass
    import concourse.tile as tile
    from concourse import bacc, mybir
    from contextlib import ExitStack

    dt = mybir.dt.float32
    dr = mybir.dt.float32r
    dh = mybir.dt.bfloat16
    d8 = mybir.dt.float8e4
    dhalf = mybir.dt.float16
    AF = mybir.ActivationFunctionType
    ALU = mybir.AluOpType
    AX = mybir.AxisListType.X

    nc = bacc.Bacc(None, num_devices=N_CORES)
    slices = nc.dram_tensor("slices", [128, nd * 1024], dh, kind="ExternalInput")
    wlp_d = nc.dram_tensor("wlp", [128, nd], dt, kind="ExternalInput")
    vbs_d = nc.dram_tensor("vbs", [128, nd * 512], dh, kind="ExternalInput")
    tents_d = nc.dram_tensor("tents", [128, nd * 512], dh, kind="ExternalInput")
    idh_d = nc.dram_tensor("identh", [128, 128], dh, kind="ExternalInput")
    idf_d = nc.dram_tensor("identf", [128, 128], dt, kind="ExternalInput")
    ones1_d = nc.dram_tensor("ones1", [1, 128], dt, kind="ExternalInput")
    out_d = nc.dram_tensor("out", [256, 256], dt, kind="ExternalOutput")
    cc_in = nc.dram_tensor("cc_in", [256, 256], dhalf)
    cc_out = nc.dram_tensor("cc_out", [256, 256], dhalf, addr_space="Shared")

    with tile.TileContext(nc) as tc, ExitStack() as ctx:
        const = ctx.enter_context(tc.tile_pool(name="const", bufs=1))
        slp = ctx.enter_context(tc.tile_pool(name="slp", bufs=4))
        work = ctx.enter_context(tc.tile_pool(name="work", bufs=3))
        epil = ctx.enter_context(tc.tile_pool(name="epil", bufs=1))
        psum = ctx.enter_context(
            tc.tile_pool(name="psum", bufs=2, space=bass.MemorySpace.PSUM))
        pst1 = ctx.enter_context(
            tc.tile_pool(name="pst1", bufs=3, space=bass.MemorySpace.PSUM))
        psacc = ctx.enter_context(
            tc.tile_pool(name="psacc", bufs=1, space=bass.MemorySpace.PSUM))

        def cload(dram, shape, dtype=dt):
            t = const.tile(shape, dtype, tag=dram.name)
            nc.sync.dma_start(t[:], dram[:])
            return t

        wlp = cload(wlp_d, [128, nd])
        identh = cload(idh_d, [128, 128], dh)
        identf = cload(idf_d, [128, 128], dt)
        ones1 = cload(ones1_d, [1, 128], dt)

        NCH = (nd + 1) // 2
        PREF = 3
        slabs = [None] * NCH
        vbsl = [None] * NCH
        tentl = [None] * NCH

        def issue_chunk(j):
            ndep = min(2, nd - 2 * j)
            t = slp.tile([128, min(2048, ndep * 1024)], dh, tag="slab")
            nc.sync.dma_start(t[:], slices[:, j * 2048:j * 2048 + t.shape[1]])
            slabs[j] = t
            v = slp.tile([128, ndep * 512], dh, tag="vbs")
            nc.sync.dma_start(v[:], vbs_d[:, j * 1024:j * 1024 + v.shape[1]])
            vbsl[j] = v
            a = slp.tile([128, ndep * 512], dh, tag="tent")
            nc.sync.dma_start(a[:], tents_d[:, j * 1024:j * 1024 + a.shape[1]])
            tentl[j] = a

        for j in range(min(PREF, NCH)):
            issue_chunk(j)

        rgbps = psacc.tile([128, 512], dt, tag="rgb")

        # software-pipelined state
        zm_t = [None] * nd      # z-merged slice tiles
        wf_t = [None] * nd      # weighted feature tiles

        def tent_ap(k):
            return tentl[k // 2][:, (k % 2) * 512:(k % 2) * 512 + 512]

        def emit_zm(k):
            j = k // 2
            base = (k % 2) * 1024
            zm = work.tile([128, 512], dh, tag="zm")
            nc.gpsimd.tensor_add(zm[:], slabs[j][:, base:base + 512],
                                 slabs[j][:, base + 512:base + 1024])
            zm_t[k] = zm

        # prologue for depth 0
        emit_zm(0)

        for k in range(nd):
            zm = zm_t[k]
            at = tent_ap(k)

            # prefetch the slab chunk PREF ahead (once per chunk)
            if k % 2 == 0 and k // 2 + PREF < NCH:
                issue_chunk(k // 2 + PREF)

            # --- mm1: T1[x,py] = sum_y Zp[y,x] * A[y,py] ---
            t1ps = pst1.tile([128, 512], dt, tag="t1")
            for xc in (0, 1):
                for yb in (0, 1):
                    nc.tensor.matmul(
                        t1ps[:, 256 * xc:256 * (xc + 1)],
                        zm[:, 256 * yb + 128 * xc:256 * yb + 128 * xc + 128],
                        at[:, 256 * yb:256 * (yb + 1)],
                        start=(yb == 0), stop=(yb == 1))

            # PE filler while ACT does the t1 copy: prev depth's rgb acc
            if k > 0:
                nc.tensor.matmul(rgbps[:], identh[:], wf_t[k - 1][:],
                                 start=(k == 1), stop=False, skip_group_check=True)

            # --- ACT: t1sb = -wz_large * T1  (PSUM->SBUF, bf16) ---
            t1sb = work.tile([128, 512], dh, tag="t1sb")
            nc.scalar.activation(t1sb[:], t1ps[:], AF.Copy, scale=wlp[:, k:k + 1])

            # --- DVE: z-merge for next depth ---
            if k + 1 < nd:
                emit_zm(k + 1)

            # --- mm2: -feat[px,py] = sum_x A[x,px] * t1sb[x,py] ---
            featps = psum.tile([128, 512], dt, tag="feat")
            for mb in (0, 1):
                for xb in (0, 1):
                    nc.tensor.matmul(
                        featps[:, 256 * mb:256 * (mb + 1)],
                        at[:, 256 * xb + 128 * mb:256 * xb + 128 * mb + 128],
                        t1sb[:, 256 * xb:256 * (xb + 1)],
                        start=(xb == 0), stop=(xb == 1))

            # --- DVE: wf = (-gamma*sigma) .* (-feat) = gamma*sigma*feat ---
            j = k // 2
            vbk = vbsl[j][:, (k % 2) * 512:(k % 2) * 512 + 512]
            wf = work.tile([128, 512], dh, tag="wf")
            nc.vector.tensor_mul(wf[:], vbk, featps[:])
            wf_t[k] = wf

        nc.tensor.matmul(rgbps[:], identh[:], wf_t[nd - 1][:],
                         start=False, stop=True, skip_group_check=True)

        # ---- cross-core reduce (fp16 AllReduce) ----
        rgbh = epil.tile([128, 512], dhalf, tag="rgbh")
        nc.vector.tensor_copy(rgbh[:], rgbps[:])
        nc.sync.dma_start(cc_in[:].rearrange("(b p) y -> p b y", p=128),
                          rgbh[:].rearrange("p (b y) -> p b y", b=2))
        if sim:
            nc.sync.dma_start(cc_out[:], cc_in[:])
        else:
            nc.gpsimd.collective_compute(
                "AllReduce", ALU.add, replica_groups=[list(range(N_CORES))],
                ins=[cc_in[:]], outs=[cc_out[:]])
        rgbfh = epil.tile([128, 512], dhalf, tag="rgbfh")
        nc.sync.dma_start(rgbfh[:].rearrange("p (b y) -> p b y", b=2),
                          cc_out[:].rearrange("(b p) y -> p b y", p=128))

        # ---- normalization ----
        # With EPS=1e-8 the standardized+renormalized composition is exactly
        # (x - min + c) / (max - min + c) with c = EPS*(std+EPS) ~ 1e-9 --
        # negligible vs the image range (~0.5), so min/max suffice (~1e-7).
        r2 = epil.tile([128, 2], dt, tag="r2")
        nc.vector.tensor_reduce(r2[:, 0:1], rgbfh[:], axis=AX, op=ALU.min)
        nc.vector.tensor_reduce(r2[:, 1:2], rgbfh[:], axis=AX, op=ALU.max)
        nc.vector.tensor_scalar_mul(r2[:, 1:2], r2[:, 1:2], -1.0)    # -max
        trt = pst1.tile([128, 512], dt, tag="t1")
        tr = trt[0:2, 0:128]
        nc.tensor.matmul(tr, r2[:], identf[:], start=True, stop=True)
        s2 = epil.tile([2, 1], dt, tag="s2")
        nc.vector.tensor_reduce(s2[0:2, 0:1], tr, axis=AX, op=ALU.min)
        g4t = pst1.tile([128, 512], dt, tag="t1")
        gm = g4t[0:1, 0:2]
        nc.tensor.matmul(gm, s2[0:2, 0:1], identf[0:2, 0:2],
                         start=True, stop=True)
        gs = epil.tile([1, 4], dt, tag="gs")
        nc.vector.tensor_copy(gs[:, 0:2], gm)                # [min, -max]
        sc2 = epil.tile([1, 2], dt, tag="sc2")
        nc.vector.tensor_add(gs[:, 2:3], gs[:, 0:1], gs[:, 1:2])     # min-max
        nc.vector.tensor_scalar_mul(gs[:, 3:4], gs[:, 2:3], -1.0)    # max-min
        nc.vector.reciprocal(sc2[:, 1:2], gs[:, 3:4])                # inv
        nc.vector.tensor_scalar_mul(sc2[:, 0:1], gs[:, 0:1], -1.0)   # -min
        # broadcast [1,2] -> [128,2]
        bct = psum.tile([128, 512], dt, tag="vb")
        bc = bct[:, 0:2]
        nc.tensor.matmul(bc, ones1[:], sc2[:], start=True, stop=True)
        bcs = epil.tile([128, 2], dt, tag="bcs")
        nc.vector.tensor_copy(bcs[:], bc)
        outsb = epil.tile([128, 512], dt, tag="outsb")
        nc.vector.tensor_scalar(outsb[:], rgbfh[:], bcs[:, 0:1], bcs[:, 1:2],
                                ALU.add, ALU.mult)
        nc.sync.dma_start(out_d[:].rearrange("(b p) y -> p b y", p=128),
                          outsb[:].rearrange("p (b y) -> p b y", b=2))
    return nc


# ----------------------------------------------------------------------------
# entry points
# ----------------------------------------------------------------------------

def _axis_aligned(R, T):
    return (np.allclose(np.asarray(R[0]), np.eye(3), atol=1e-6)
            and abs(float(T[0][0]) - float(T[0][1])) < 1e-12)


class _CachedSpmd:
    """Compile the PJRT executable once; repeat calls only transfer + exec."""

    def __init__(self, nc, n_cores):
        import jax
        from concourse import mybir
        from concourse.bass2jax import (_bass_exec_p, install_neuronx_cc_hook,
                                        partition_id_tensor)
        from jax.experimental.shard_map import shard_map
        from jax.sharding import Mesh, PartitionSpec
        install_neuronx_cc_hook()
        self.jax = jax
        self.n_cores = n_cores
        pname = nc.partition_id_tensor.name if nc.partition_id_tensor else None
        in_names, out_names, out_avals, zero_outs = [], [], [], []
        for alloc in nc.m.functions[0].allocations:
            if not isinstance(alloc, mybir.MemoryLocationSet):
                continue
            name = alloc.memorylocations[0].name
            if alloc.kind == "ExternalInput":
                if name != pname:
                    in_names.append(name)
            elif alloc.kind == "ExternalOutput":
                shape = tuple(alloc.tensor_shape)
                dtype = mybir.dt.np(alloc.dtype)
                out_names.append(name)
                out_avals.append(jax.core.ShapedArray(shape, dtype))
                zero_outs.append(np.zeros(shape, dtype))
        self.in_names, self.out_names = in_names, out_names
        self.out_avals, self.zero_outs = out_avals, zero_outs
        n_params, n_outs = len(in_names), len(out_names)
        all_in = list(in_names) + list(out_names)
        if pname is not None:
            all_in.append(pname)

        def _body(*args):
            operands = list(args)
            if pname is not None:
                operands.append(partition_id_tensor())
            outs = _bass_exec_p.bind(
                *operands, out_avals=tuple(out_avals), in_names=tuple(all_in),
                out_names=tuple(out_names), lowering_input_output_aliases=(),
                sim_require_finite=True, sim_require_nnan=True, nc=nc)
            return tuple(outs)

        devices = jax.devices()[:n_cores]
        mesh = Mesh(np.asarray(devices), ("core",))
        in_specs = (PartitionSpec("core"),) * (n_params + n_outs)
        out_specs = (PartitionSpec("core"),) * n_outs
        self.fn = jax.jit(shard_map(_body, mesh=mesh, in_specs=in_specs,
                                    out_specs=out_specs, check_rep=False),
                          keep_unused=True)
        self._dev_zeros = [jax.device_put(np.zeros(
            (n_cores * z.shape[0], *z.shape[1:]), z.dtype)) for z in zero_outs]

    def run(self, in_maps):
        jax = self.jax
        concat = [np.concatenate([np.asarray(in_maps[c][nm])
                                  for c in range(self.n_cores)], axis=0)
                  for nm in self.in_names]
        outs = self.fn(*concat, *self._dev_zeros)
        jax.block_until_ready(outs)
        return [{nm: np.asarray(outs[i]).reshape(
                    self.n_cores, *self.out_avals[i].shape)[c]
                 for i, nm in enumerate(self.out_names)}
                for c in range(self.n_cores)]


_RUNNER_CACHE = {}


def _run(image3d, R, T, trace=False):
    vol = np.ascontiguousarray(np.asarray(image3d, np.float32)[0, 0])
    in_maps, nd = _host_inputs(vol, np.asarray(T, np.float64)[0])
    if nd not in _NC_CACHE:
        nc = _build_nc(nd)
        nc.finalize()
        _NC_CACHE[nd] = nc
    nc = _NC_CACHE[nd]
    if id(nc) not in _RUNNER_CACHE:
        _RUNNER_CACHE[id(nc)] = _CachedSpmd(nc, N_CORES)
    results = _RUNNER_CACHE[id(nc)].run(in_maps)
    out = np.asarray(results[0]["out"], np.float32)[None, None]
    return out, results


def _numpy_fallback(image3d, R, T):
    """Direct port of the reference for non-axis-aligned cameras."""
    image3d = np.asarray(image3d, np.float32)
    R = np.asarray(R, np.float32); T = np.asarray(T, np.float32)
    B, C, D, H, W = image3d.shape
    vol = image3d[:, 0]
    vox = 3.0 / max(C, D)
    yg, xg = np.meshgrid(np.linspace(-1, 1, IMG), np.linspace(-1, 1, IMG),
                         indexing='ij')
    depths = np.linspace(MIN_D, MAX_D, NPTS)
    pcam = np.stack([xg[..., None] * depths / FOCAL,
                     yg[..., None] * depths / FOCAL,
                     np.broadcast_to(depths, (IMG, IMG, NPTS))], -1)
    v = pcam[None] - T[:, None, None, None, :]
    pw = np.einsum('bhwpj,bkj->bhwpk', v, R)
    half = np.array([vox * (W - 1) / 2, vox * (H - 1) / 2, vox * (D - 1) / 2])
    local = pw / half

    def tri(voln, pts):
        ix = (pts[..., 0] + 1) * .5 * (W - 1)
        iy = (pts[..., 1] + 1) * .5 * (H - 1)
        iz = (pts[..., 2] + 1) * .5 * (D - 1)
        out = np.zeros(ix.shape, np.float32)
        x0, y0, z0 = np.floor(ix), np.floor(iy), np.floor(iz)
        fx, fy, fz = ix - x0, iy - y0, iz - z0
        for zi, wz in ((z0, 1 - fz), (z0 + 1, fz)):
            for yi, wy in ((y0, 1 - fy), (y0 + 1, fy)):
                for xi, wx in ((x0, 1 - fx), (x0 + 1, fx)):
                    valid = ((xi >= 0) & (xi < W) & (yi >= 0) & (yi < H)
                             & (zi >= 0) & (zi < D))
                    vv = voln[np.clip(zi, 0, D - 1).astype(int),
                              np.clip(yi, 0, H - 1).astype(int),
                              np.clip(xi, 0, W - 1).astype(int)]
                    out += np.where(valid, vv * (wz * wy * wx), 0).astype(np.float32)
        return out

    feat = np.stack([tri(vol[b], local[b]) for b in range(B)])
    sigma = DENSITY * np.stack([tri(np.ones((D, H, W), np.float32), local[b])
                                for b in range(B)])
    t = (1.0 + 1e-10) - sigma
    ab = np.cumprod(t, -1)
    ab = np.concatenate([np.ones_like(ab[..., :1]), ab[..., :-1]], -1)
    rgb = np.sum(sigma * ab * feat, -1)
    out = np.transpose(rgb, (0, 2, 1))[:, None]
    s = (out - out.mean()) / (np.std(out, ddof=1) + EPS)
    return ((s - s.min() + EPS) / (s.max() - s.min() + EPS)).astype(np.float32)


def kernel(image3d, R, T):
    if not _axis_aligned(R, T):
        return _numpy_fallback(image3d, R, T)
    out, _ = _run(image3d, R, T, trace=False)
    return out
